# revision 1
# baseline (speedup 1.0000x reference)
# Trainium2 Bass kernel for nn_LNKillingRelu: out = where(kf<=0, x, x + kf*d)
#   d  = einsum('fkn,gf->gkn', x, W)                      (per batch)
#   kf = einsum('fkn,kl,fln->fn', x, G, d)  broadcast over k
# G is the (constant) Killing-form Gram matrix of sl(3):
#   G[0,0]=G[4,4]=12, G[0,4]=G[4,0]=-6, G[1,3]=G[3,1]=G[2,6]=G[6,2]=G[5,7]=G[7,5]=6
# so with kf' = kf/6:
#   kf' = x0*(2d0-d4) + x4*(2d4-d0) + x1*d3 + x3*d1 + x2*d6 + x6*d2 + x5*d7 + x7*d5
#   out = x + relu(6*kf') * d
#
# Sharding: data-parallel over batch B=8 -> one batch per NeuronCore (8 cores).
# W is replicated (host passes W^T so lhsT chunks slice directly).

from contextlib import ExitStack

import numpy as np

import concourse.bass as bass
import concourse.mybir as mybir
import concourse.tile as tile
from concourse.bass_utils import run_bass_kernel_spmd

B, F, K, N = 8, 512, 8, 2048
P = 128
FT = F // P  # 4 channel tiles

f32 = mybir.dt.float32
Alu = mybir.AluOpType
ActF = mybir.ActivationFunctionType


def _ap(base, off_elems, dims):
    """Raw AP from a base AP: keep partition dim, replace free dims."""
    return bass.AP(
        tensor=base.tensor,
        offset=base.offset + off_elems,
        ap=[base.ap[0]] + dims,
    )


def build_nc(n_total=N, nt=256):
    nch = n_total // nt
    # race detection chokes on the post-hoc wait-split NoOps (they lack the
    # rust pass's fake sem updates); correctness was validated in CoreSim.
    nc = bass.Bass(detect_race_conditions=False)
    x = nc.dram_tensor("x", [F, K, n_total], f32, kind="ExternalInput")
    wt = nc.dram_tensor("wt", [F, F], f32, kind="ExternalInput")  # W^T (f, g)
    out = nc.dram_tensor("out", [F, K, n_total], f32, kind="ExternalOutput")

    with TileContextCompat(nc) as tc, ExitStack() as ctx:
        wpool = ctx.enter_context(tc.tile_pool(name="w", bufs=1))
        xpool = ctx.enter_context(tc.tile_pool(name="xc", bufs=2))
        ppool = ctx.enter_context(tc.tile_pool(name="pd", bufs=2, space="PSUM"))
        prpool = ctx.enter_context(tc.tile_pool(name="prod", bufs=2))
        spool = ctx.enter_context(tc.tile_pool(name="small", bufs=3))
        opool = ctx.enter_context(tc.tile_pool(name="og", bufs=3))

        # resident W^T tiles: wsb[ft][p, g] , f = ft*128+p
        wsb = []
        for ft in range(FT):
            w_t = wpool.tile([P, F], f32, tag=f"w{ft}")
            nc.sync.dma_start(out=w_t[:], in_=wt[ft * P : (ft + 1) * P, :])
            wsb.append(w_t)

        # Walrus only allows ONE sync wait per Matmult (waits ride the
        # LDWEIGHTS struct).  Warmup matmuls make PE observe each W-DMA
        # semaphore individually so later matmuls never wait on W.
        warm = ppool.tile([P, K, nt], f32, tag="pd")
        for ft in range(FT):
            nc.tensor.matmul(
                warm[:, 0, 0:1], wsb[ft][:, 0:P], wsb[ft][:, 0:1], start=True, stop=True
            )

        for c in range(nch):
            xcs = []
            for ft in range(FT):
                xt = xpool.tile([P, K, nt], f32, tag=f"xc{ft}")
                nc.sync.dma_start(
                    out=xt[:],
                    in_=x[ft * P : (ft + 1) * P, :, c * nt : (c + 1) * nt],
                )
                xcs.append(xt)
            for gt in range(FT):
                # ---- matmul: d[g, k, n-chunk] accumulated over f tiles ----
                pd = ppool.tile([P, K, nt], f32, tag="pd")
                # Dummy first matmul absorbs the PSUM-slot-release wait so the
                # first real matmul only waits on its x DMA (1-wait limit).
                nc.tensor.matmul(
                    pd[:, 0, 0:1], wsb[0][:, 0:P], wsb[0][:, 0:1], start=True, stop=True
                )
                nmm = (K * nt) // 512  # 512-elem free chunks (one PSUM bank each)
                kper = 512 // nt  # k planes per matmul chunk
                # ft outer: same lhsT for nmm consecutive matmuls (weight reuse)
                for ft in range(FT):
                    for jj in range(nmm):
                        nc.tensor.matmul(
                            pd[:, jj * kper : (jj + 1) * kper, :],
                            wsb[ft][:, gt * P : (gt + 1) * P],
                            xcs[ft][:, jj * kper : (jj + 1) * kper, :],
                            start=(ft == 0),
                            stop=(ft == FT - 1),
                        )

                xg = xcs[gt][:]  # [P, K, nt] x values for this channel tile
                pdb = pd[:]

                # kf' = sum_l z_l * d_l with z = (G/6) applied to x along k
                # (G symmetric; keeps all permuted/fused reads on SBUF since
                #  walrus only allows ONE PSUM operand per instruction).
                # ---- aux = (2x0-x4, 2x4-x0) : one fused STT op, all SBUF ----
                aux = spool.tile([P, 2, nt], f32, tag="aux")
                nc.vector.scalar_tensor_tensor(
                    out=aux[:],
                    in0=xg[:, 0::4, :],  # x0, x4
                    scalar=2.0,
                    in1=xg[:, 4::-4, :],  # x4, x0
                    op0=Alu.mult,
                    op1=Alu.subtract,
                )

                # ---- products p_l = z_l * d_l (3 ops, G-sparsity) ----
                p = prpool.tile([P, K, nt], f32, tag="p")
                # l in (1,3,5,7): z_l = x at (3,1,7,5)
                nc.vector.tensor_tensor(
                    out=_ap(p[:], nt, [[4 * nt, 2], [2 * nt, 2], [1, nt]]),
                    in0=_ap(xg, 3 * nt, [[4 * nt, 2], [-2 * nt, 2], [1, nt]]),
                    in1=_ap(pdb, nt, [[4 * nt, 2], [2 * nt, 2], [1, nt]]),
                    op=Alu.mult,
                )
                # l in (2,6): z_l = x at (6,2)
                nc.vector.tensor_tensor(
                    out=p[:, 2::4, :],
                    in0=xg[:, 6::-4, :],
                    in1=pd[:, 2::4, :],
                    op=Alu.mult,
                )
                # l in (0,4): z_l = aux
                nc.vector.tensor_tensor(
                    out=p[:, 0::4, :],
                    in0=aux[:],
                    in1=pd[:, 0::4, :],
                    op=Alu.mult,
                )

                # ---- kf' = sum_k p_k  (reduce innermost k of [P, n, k] view) ----
                kf = spool.tile([P, nt], f32, tag="kf")
                nc.vector.tensor_reduce(
                    out=kf[:],
                    in_=_ap(p[:], 0, [[1, nt], [nt, K]]),
                    axis=mybir.AxisListType.X,
                    op=Alu.add,
                )

                # ---- gate = relu(6 * kf') on ScalarE ----
                gate = spool.tile([P, nt], f32, tag="gate")
                nc.scalar.activation(
                    out=gate[:], in_=kf[:], func=ActF.Relu, scale=6.0
                )

                # ---- og = gate (bcast over k) * d ----
                og = opool.tile([P, K, nt], f32, tag="og")
                nc.vector.tensor_tensor(
                    out=og[:],
                    in0=_ap(gate[:], 0, [[0, K], [1, nt]]),
                    in1=pdb,
                    op=Alu.mult,
                )

                # ---- og += x on GpSimd (keeps DVE free) ----
                nc.gpsimd.tensor_tensor(out=og[:], in0=og[:], in1=xg, op=Alu.add)

                nc.sync.dma_start(
                    out=out[gt * P : (gt + 1) * P, :, c * nt : (c + 1) * nt],
                    in_=og[:],
                )

    _split_waits(nc)
    return nc


# Engine datapath structs (Matmult/TT/STT/Act/...) only carry ONE sync wait on
# TRN2 walrus; sequencer instructions (NoOp) can each carry one more.  Hoist
# surplus waits onto same-engine NoOps placed just before the instruction.
_SEQ_OK = set()  # every struct on this walrus takes at most ONE sync wait


def _split_waits(nc):
    nnop = 0
    for fn in nc.m.functions:
        for blk in fn.blocks:
            out = []
            for inst in blk.instructions:
                si = inst.sync_info
                if (
                    si is not None
                    and si.on_wait
                    and len(si.on_wait) > 1
                    and type(inst).__name__ not in _SEQ_OK
                ):
                    for w in si.on_wait[:-1]:
                        nop = mybir.InstNoOp(
                            name=f"{inst.name}-sw{nnop}",
                            opcode="NoOp",
                            engine=inst.engine,
                            sync_info=mybir.SyncInfo(on_wait=[w], on_update=[]),
                        )
                        nnop += 1
                        out.append(nop)
                    inst.sync_info = mybir.SyncInfo(
                        on_wait=[si.on_wait[-1]], on_update=list(si.on_update)
                    )
                out.append(inst)
            blk.instructions[:] = out
    return nc


def TileContextCompat(nc):
    return tile.TileContext(nc)


_NC_CACHE = {}


def _get_nc(n_total=N, nt=256):
    key = (n_total, nt)
    if key not in _NC_CACHE:
        _NC_CACHE[key] = build_nc(n_total, nt)
    return _NC_CACHE[key]


def kernel(x: np.ndarray, W: np.ndarray) -> np.ndarray:
    assert x.shape == (B, F, K, N) and W.shape == (F, F)
    wt = np.ascontiguousarray(W.T.astype(np.float32))
    in_maps = [
        {"x": np.ascontiguousarray(x[b].astype(np.float32)), "wt": wt}
        for b in range(B)
    ]
    nc = _get_nc()
    res = run_bass_kernel_spmd(nc, in_maps, list(range(B)))
    return np.stack([res.results[b]["out"] for b in range(B)], axis=0)


if __name__ == "__main__":
    xs = np.random.randn(B, F, K, N).astype(np.float32)
    Ws = (np.random.randn(F, F) / np.sqrt(F)).astype(np.float32)
    o = kernel(xs, Ws)
    print(o.shape, o.dtype)



# revision 3
# speedup vs baseline: 1.2268x; 1.2268x over previous
# Trainium2 Bass kernel for nn_LNKillingRelu: out = where(kf<=0, x, x + kf*d)
#   d  = einsum('fkn,gf->gkn', x, W)                      (per batch)
#   kf = einsum('fkn,kl,fln->fn', x, G, d)  broadcast over k
# G is the (constant) Killing-form Gram matrix of sl(3):
#   G[0,0]=G[4,4]=12, G[0,4]=G[4,0]=-6, G[1,3]=G[3,1]=G[2,6]=G[6,2]=G[5,7]=G[7,5]=6
# so with kf' = kf/6:
#   kf' = x0*(2d0-d4) + x4*(2d4-d0) + x1*d3 + x3*d1 + x2*d6 + x6*d2 + x5*d7 + x7*d5
#   out = x + relu(6*kf') * d
#
# v2: full fp16 datapath (validated: rel err ~1.2e-3 vs fp32 reference).
#  - fp16 matmul: 1 PE cycle/row vs fp32's 4 -> 4x tensor throughput
#  - all element-wise ops as scalar_tensor_tensor (InstTensorScalarPtr),
#    which supports the DVE 4x_2p perf mode (2-byte packed operands, all
#    SBUF) -> 0.25 cycles/elem
#  - d copied PSUM(fp32)->SBUF(fp16) on the idle Scalar engine so DVE ops
#    keep all operands in SBUF at 2 bytes
#  - fp16 I/O halves DMA traffic (host converts fp32<->fp16)
#
# Sharding: data-parallel over batch B=8 -> one batch per NeuronCore (8 cores).
# W is replicated (host passes W^T in fp16 so lhsT chunks slice directly).

from contextlib import ExitStack

import numpy as np

import concourse.bass as bass
import concourse.mybir as mybir
import concourse.tile as tile
from concourse.bass_utils import run_bass_kernel_spmd

B, F, K, N = 8, 512, 8, 2048
P = 128
FT = F // P  # 4 channel tiles

f32 = mybir.dt.float32
f16 = mybir.dt.float16
Alu = mybir.AluOpType
ActF = mybir.ActivationFunctionType


def _ap(base, off_elems, dims):
    """Raw AP from a base AP: keep partition dim, replace free dims."""
    return bass.AP(
        tensor=base.tensor,
        offset=base.offset + off_elems,
        ap=[base.ap[0]] + dims,
    )


def build_nc(n_total=N, nt=256):
    nch = n_total // nt
    # race detection chokes on the post-hoc wait-split NoOps (they lack the
    # rust pass's fake sem updates); correctness is validated vs reference.
    nc = bass.Bass(detect_race_conditions=False)
    x = nc.dram_tensor("x", [F, K, n_total], f16, kind="ExternalInput")
    wt = nc.dram_tensor("wt", [F, F], f16, kind="ExternalInput")  # W^T (f, g)
    out = nc.dram_tensor("out", [F, K, n_total], f16, kind="ExternalOutput")

    with tile.TileContext(nc) as tc, ExitStack() as ctx:
        wpool = ctx.enter_context(tc.tile_pool(name="w", bufs=1))
        xpool = ctx.enter_context(tc.tile_pool(name="xc", bufs=2))
        ppool = ctx.enter_context(tc.tile_pool(name="pd", bufs=2, space="PSUM"))
        dcpool = ctx.enter_context(tc.tile_pool(name="dc", bufs=3))
        prpool = ctx.enter_context(tc.tile_pool(name="prod", bufs=2))
        spool = ctx.enter_context(tc.tile_pool(name="small", bufs=3))
        opool = ctx.enter_context(tc.tile_pool(name="og", bufs=3))

        # resident W^T tiles: wsb[ft][p, g] , f = ft*128+p
        wsb = []
        for ft in range(FT):
            w_t = wpool.tile([P, F], f16, tag=f"w{ft}")
            nc.sync.dma_start(out=w_t[:], in_=wt[ft * P : (ft + 1) * P, :])
            wsb.append(w_t)

        # Walrus only allows ONE sync wait per Matmult (waits ride the
        # LDWEIGHTS struct).  Warmup matmuls make PE observe each W-DMA
        # semaphore individually so later matmuls never wait on W.
        warm = ppool.tile([P, K, nt], f32, tag="pd")
        for ft in range(FT):
            nc.tensor.matmul(
                warm[:, 0, 0:1], wsb[ft][:, 0:P], wsb[ft][:, 0:1], start=True, stop=True
            )

        for c in range(nch):
            xcs = []
            for ft in range(FT):
                xt = xpool.tile([P, K, nt], f16, tag=f"xc{ft}")
                nc.sync.dma_start(
                    out=xt[:],
                    in_=x[ft * P : (ft + 1) * P, :, c * nt : (c + 1) * nt],
                )
                xcs.append(xt)
            for gt in range(FT):
                # ---- matmul: d[g, k, n-chunk] accumulated over f tiles ----
                pd = ppool.tile([P, K, nt], f32, tag="pd")
                # Dummy first matmul absorbs the PSUM-slot-release wait so the
                # first real matmul only waits on its x DMA (1-wait limit).
                nc.tensor.matmul(
                    pd[:, 0, 0:1], wsb[0][:, 0:P], wsb[0][:, 0:1], start=True, stop=True
                )
                nmm = (K * nt) // 512  # 512-elem free chunks (one PSUM bank each)
                kper = 512 // nt  # k planes per matmul chunk
                # ft outer: same lhsT for nmm consecutive matmuls (weight reuse)
                for ft in range(FT):
                    for jj in range(nmm):
                        nc.tensor.matmul(
                            pd[:, jj * kper : (jj + 1) * kper, :],
                            wsb[ft][:, gt * P : (gt + 1) * P],
                            xcs[ft][:, jj * kper : (jj + 1) * kper, :],
                            start=(ft == 0),
                            stop=(ft == FT - 1),
                        )

                xg = xcs[gt][:]  # [P, K, nt] fp16 x values for this channel tile

                # ---- dc = fp16(d): PSUM -> SBUF on the (otherwise idle)
                # Scalar engine, so every DVE op below is all-SBUF 2-byte
                # (the 4x_2p perf-mode requirement). ----
                dc = dcpool.tile([P, K, nt], f16, tag="dc")
                nc.scalar.copy(out=dc[:], in_=pd[:])
                dcb = dc[:]

                # kf' = sum_l z_l * d_l with z = (G/6) applied to x along k.
                # ---- aux = (2x0-x4, 2x4-x0) : one fused STT op ----
                aux = spool.tile([P, 2, nt], f16, tag="aux")
                nc.vector.scalar_tensor_tensor(
                    out=aux[:],
                    in0=xg[:, 0::4, :],  # x0, x4
                    scalar=2.0,
                    in1=xg[:, 4::-4, :],  # x4, x0
                    op0=Alu.mult,
                    op1=Alu.subtract,
                )

                # ---- products p_l = z_l * d_l (4 STT ops, G-sparsity;
                # walrus caps TensorScalarPtr APs at 2 free dims) ----
                p = prpool.tile([P, K, nt], f16, tag="p")
                # l in (1,5): z_l = x at (3,7)
                nc.vector.scalar_tensor_tensor(
                    out=_ap(p[:], nt, [[4 * nt, 2], [1, nt]]),
                    in0=_ap(xg, 3 * nt, [[4 * nt, 2], [1, nt]]),
                    scalar=1.0,
                    in1=_ap(dcb, nt, [[4 * nt, 2], [1, nt]]),
                    op0=Alu.mult,
                    op1=Alu.mult,
                )
                # l in (3,7): z_l = x at (1,5)
                nc.vector.scalar_tensor_tensor(
                    out=_ap(p[:], 3 * nt, [[4 * nt, 2], [1, nt]]),
                    in0=_ap(xg, nt, [[4 * nt, 2], [1, nt]]),
                    scalar=1.0,
                    in1=_ap(dcb, 3 * nt, [[4 * nt, 2], [1, nt]]),
                    op0=Alu.mult,
                    op1=Alu.mult,
                )
                # l in (2,6): z_l = x at (6,2)
                nc.vector.scalar_tensor_tensor(
                    out=p[:, 2::4, :],
                    in0=xg[:, 6::-4, :],
                    scalar=1.0,
                    in1=dc[:, 2::4, :],
                    op0=Alu.mult,
                    op1=Alu.mult,
                )
                # l in (0,4): z_l = aux
                nc.vector.scalar_tensor_tensor(
                    out=p[:, 0::4, :],
                    in0=aux[:],
                    scalar=1.0,
                    in1=dc[:, 0::4, :],
                    op0=Alu.mult,
                    op1=Alu.mult,
                )

                # ---- kf' = sum_k p_k as a binary tree of STT adds (the DVE
                # TensorReduce op has no fast mode; STT adds run at 4x) ----
                t1 = spool.tile([P, 4, nt], f16, tag="t1")
                nc.vector.scalar_tensor_tensor(
                    out=t1[:], in0=p[:, 0:4, :], scalar=1.0, in1=p[:, 4:8, :],
                    op0=Alu.mult, op1=Alu.add,
                )
                t2 = spool.tile([P, 2, nt], f16, tag="t2")
                nc.vector.scalar_tensor_tensor(
                    out=t2[:], in0=t1[:, 0:2, :], scalar=1.0, in1=t1[:, 2:4, :],
                    op0=Alu.mult, op1=Alu.add,
                )
                kf = spool.tile([P, nt], f16, tag="kf")
                nc.vector.scalar_tensor_tensor(
                    out=kf[:], in0=t2[:, 0, :], scalar=1.0, in1=t2[:, 1, :],
                    op0=Alu.mult, op1=Alu.add,
                )

                # ---- gate = relu(6 * kf') on ScalarE ----
                gate = spool.tile([P, nt], f16, tag="gate")
                nc.scalar.activation(
                    out=gate[:], in_=kf[:], func=ActF.Relu, scale=6.0
                )

                # ---- og = gate (bcast over k) * d ----
                og = opool.tile([P, K, nt], f16, tag="og")
                nc.vector.scalar_tensor_tensor(
                    out=og[:],
                    in0=dcb,
                    scalar=1.0,
                    in1=_ap(gate[:], 0, [[0, K], [1, nt]]),
                    op0=Alu.mult,
                    op1=Alu.mult,
                )

                # ---- o2 = og + x ----
                o2 = opool.tile([P, K, nt], f16, tag="o2")
                nc.vector.scalar_tensor_tensor(
                    out=o2[:], in0=og[:], scalar=1.0, in1=xg,
                    op0=Alu.mult, op1=Alu.add,
                )

                nc.sync.dma_start(
                    out=out[gt * P : (gt + 1) * P, :, c * nt : (c + 1) * nt],
                    in_=o2[:],
                )

    _split_waits(nc)
    return nc


# Engine datapath structs (Matmult/TT/STT/Act/...) only carry ONE sync wait on
# TRN2 walrus; sequencer instructions (NoOp) can each carry one more.  Hoist
# surplus waits onto same-engine NoOps placed just before the instruction.
_SEQ_OK = set()  # every struct on this walrus takes at most ONE sync wait


def _split_waits(nc):
    nnop = 0
    for fn in nc.m.functions:
        for blk in fn.blocks:
            out = []
            for inst in blk.instructions:
                si = inst.sync_info
                if (
                    si is not None
                    and si.on_wait
                    and len(si.on_wait) > 1
                    and type(inst).__name__ not in _SEQ_OK
                ):
                    for w in si.on_wait[:-1]:
                        nop = mybir.InstNoOp(
                            name=f"{inst.name}-sw{nnop}",
                            opcode="NoOp",
                            engine=inst.engine,
                            sync_info=mybir.SyncInfo(on_wait=[w], on_update=[]),
                        )
                        nnop += 1
                        out.append(nop)
                    inst.sync_info = mybir.SyncInfo(
                        on_wait=[si.on_wait[-1]], on_update=list(si.on_update)
                    )
                out.append(inst)
            blk.instructions[:] = out
    return nc


_NC_CACHE = {}


def _get_nc(n_total=N, nt=256):
    key = (n_total, nt)
    if key not in _NC_CACHE:
        _NC_CACHE[key] = build_nc(n_total, nt)
    return _NC_CACHE[key]


def kernel(x: np.ndarray, W: np.ndarray) -> np.ndarray:
    assert x.shape == (B, F, K, N) and W.shape == (F, F)
    wt = np.ascontiguousarray(W.T.astype(np.float16))
    x16 = x.astype(np.float16)
    in_maps = [
        {"x": np.ascontiguousarray(x16[b]), "wt": wt}
        for b in range(B)
    ]
    nc = _get_nc()
    res = run_bass_kernel_spmd(nc, in_maps, list(range(B)))
    return np.stack(
        [res.results[b]["out"].astype(np.float32) for b in range(B)], axis=0
    )


if __name__ == "__main__":
    xs = np.random.randn(B, F, K, N).astype(np.float32)
    Ws = (np.random.randn(F, F) / np.sqrt(F)).astype(np.float32)
    o = kernel(xs, Ws)
    print(o.shape, o.dtype)


# revision 4
# speedup vs baseline: 1.8733x; 1.5270x over previous
# Trainium2 Bass kernel for nn_LNKillingRelu: out = where(kf<=0, x, x + kf*d)
#   d  = einsum('fkn,gf->gkn', x, W)                      (per batch)
#   kf = einsum('fkn,kl,fln->fn', x, G, d)  broadcast over k
# G is the (constant) Killing-form Gram matrix of sl(3):
#   G[0,0]=G[4,4]=12, G[0,4]=G[4,0]=-6, G[1,3]=G[3,1]=G[2,6]=G[6,2]=G[5,7]=G[7,5]=6
# so with kf' = kf/6:
#   kf' = x0*(2d0-d4) + x4*(2d4-d0) + x1*d3 + x3*d1 + x2*d6 + x6*d2 + x5*d7 + x7*d5
#   out = x + relu(6*kf') * d
#
# v3: bf16 datapath tuned to measured per-op HW rates (validated numerically:
# rel err ~7e-3 vs fp32 reference; harness gate 2e-2):
#  - bf16 matmul: 1 PE cycle/row (216 ns / 512-row matmul measured) -> 4x fp32
#  - DVE tensor_tensor bf16 MULT runs in 4x mode (692 ns @ FD2048 measured);
#    bf16 ADD/SUB runs 2x; STT runs 1x -> products/og use TT-mult, tree uses
#    TT-add, aux's 2*x scale moved to ScalarE (activation scale), final k-sum
#    and most of the +x adds moved to GpSimd to keep DVE under the PE wall
#  - d copied PSUM(fp32)->SBUF(bf16) on ScalarE so DVE ops are all-SBUF 2-byte
#  - bf16 I/O halves DMA traffic (host converts fp32<->bf16)
#
# Sharding: data-parallel over batch B=8 -> one batch per NeuronCore (8 cores).
# W is replicated (host passes W^T in bf16 so lhsT chunks slice directly).

from contextlib import ExitStack

import numpy as np

import concourse.bass as bass
import concourse.mybir as mybir
import concourse.tile as tile
from concourse.bass_utils import run_bass_kernel_spmd

B, F, K, N = 8, 512, 8, 2048
P = 128
FT = F // P  # 4 channel tiles

f32 = mybir.dt.float32
bf16 = mybir.dt.bfloat16
Alu = mybir.AluOpType
ActF = mybir.ActivationFunctionType


def _ap(base, off_elems, dims):
    """Raw AP from a base AP: keep partition dim, replace free dims."""
    return bass.AP(
        tensor=base.tensor,
        offset=base.offset + off_elems,
        ap=[base.ap[0]] + dims,
    )


def build_nc(n_total=N, nt=256):
    nch = n_total // nt
    # race detection chokes on the post-hoc wait-split NoOps (they lack the
    # rust pass's fake sem updates); correctness is validated vs reference.
    nc = bass.Bass(detect_race_conditions=False)
    x = nc.dram_tensor("x", [F, K, n_total], bf16, kind="ExternalInput")
    wt = nc.dram_tensor("wt", [F, F], bf16, kind="ExternalInput")  # W^T (f, g)
    out = nc.dram_tensor("out", [F, K, n_total], bf16, kind="ExternalOutput")

    with tile.TileContext(nc) as tc, ExitStack() as ctx:
        wpool = ctx.enter_context(tc.tile_pool(name="w", bufs=1))
        xpool = ctx.enter_context(tc.tile_pool(name="xc", bufs=2))
        ppool = ctx.enter_context(tc.tile_pool(name="pd", bufs=2, space="PSUM"))
        dcpool = ctx.enter_context(tc.tile_pool(name="dc", bufs=3))
        prpool = ctx.enter_context(tc.tile_pool(name="prod", bufs=2))
        spool = ctx.enter_context(tc.tile_pool(name="small", bufs=3))
        opool = ctx.enter_context(tc.tile_pool(name="og", bufs=3))

        # resident W^T tiles: wsb[ft][p, g] , f = ft*128+p
        wsb = []
        for ft in range(FT):
            w_t = wpool.tile([P, F], bf16, tag=f"w{ft}")
            nc.sync.dma_start(out=w_t[:], in_=wt[ft * P : (ft + 1) * P, :])
            wsb.append(w_t)

        # Walrus only allows ONE sync wait per Matmult (waits ride the
        # LDWEIGHTS struct).  Warmup matmuls make PE observe each W-DMA
        # semaphore individually so later matmuls never wait on W.
        warm = ppool.tile([P, K, nt], f32, tag="pd")
        for ft in range(FT):
            nc.tensor.matmul(
                warm[:, 0, 0:1], wsb[ft][:, 0:P], wsb[ft][:, 0:1], start=True, stop=True
            )

        for c in range(nch):
            xcs = []
            for ft in range(FT):
                xt = xpool.tile([P, K, nt], bf16, tag=f"xc{ft}")
                nc.sync.dma_start(
                    out=xt[:],
                    in_=x[ft * P : (ft + 1) * P, :, c * nt : (c + 1) * nt],
                )
                xcs.append(xt)
            for gt in range(FT):
                # ---- matmul: d[g, k, n-chunk] accumulated over f tiles ----
                pd = ppool.tile([P, K, nt], f32, tag="pd")
                # Dummy first matmul absorbs the PSUM-slot-release wait so the
                # first real matmul only waits on its x DMA (1-wait limit).
                nc.tensor.matmul(
                    pd[:, 0, 0:1], wsb[0][:, 0:P], wsb[0][:, 0:1], start=True, stop=True
                )
                nmm = (K * nt) // 512  # 512-elem free chunks (one PSUM bank each)
                kper = 512 // nt  # k planes per matmul chunk
                # ft outer: same lhsT for nmm consecutive matmuls (weight reuse)
                for ft in range(FT):
                    for jj in range(nmm):
                        nc.tensor.matmul(
                            pd[:, jj * kper : (jj + 1) * kper, :],
                            wsb[ft][:, gt * P : (gt + 1) * P],
                            xcs[ft][:, jj * kper : (jj + 1) * kper, :],
                            start=(ft == 0),
                            stop=(ft == FT - 1),
                        )

                xg = xcs[gt][:]  # [P, K, nt] bf16 x values for this channel tile

                # ---- dc = bf16(d): PSUM -> SBUF on ScalarE, so DVE ops below
                # are all-SBUF 2-byte (fast perf-mode requirement). ----
                dc = dcpool.tile([P, K, nt], bf16, tag="dc")
                nc.scalar.copy(out=dc[:], in_=pd[:])
                dcb = dc[:]

                # ---- x2 = 2*(x0, x4) on ScalarE (exact in bf16) ----
                x2 = spool.tile([P, 2, nt], bf16, tag="x2")
                nc.scalar.activation(
                    out=x2[:], in_=xg[:, 0::4, :], func=ActF.Copy, scale=2.0
                )

                # ---- aux = (2x0-x4, 2x4-x0): one DVE TT subtract (2x) ----
                aux = spool.tile([P, 2, nt], bf16, tag="aux")
                nc.vector.tensor_tensor(
                    out=aux[:], in0=x2[:], in1=xg[:, 4::-4, :], op=Alu.subtract
                )

                # ---- products p_l = z_l * d_l (4 TT-mults, 4x mode) ----
                p = prpool.tile([P, K, nt], bf16, tag="p")
                # l in (1,5): z_l = x at (3,7)
                nc.vector.tensor_tensor(
                    out=_ap(p[:], nt, [[4 * nt, 2], [1, nt]]),
                    in0=_ap(xg, 3 * nt, [[4 * nt, 2], [1, nt]]),
                    in1=_ap(dcb, nt, [[4 * nt, 2], [1, nt]]),
                    op=Alu.mult,
                )
                # l in (3,7): z_l = x at (1,5)
                nc.vector.tensor_tensor(
                    out=_ap(p[:], 3 * nt, [[4 * nt, 2], [1, nt]]),
                    in0=_ap(xg, nt, [[4 * nt, 2], [1, nt]]),
                    in1=_ap(dcb, 3 * nt, [[4 * nt, 2], [1, nt]]),
                    op=Alu.mult,
                )
                # l in (2,6): z_l = x at (6,2)
                nc.vector.tensor_tensor(
                    out=p[:, 2::4, :],
                    in0=xg[:, 6::-4, :],
                    in1=dc[:, 2::4, :],
                    op=Alu.mult,
                )
                # l in (0,4): z_l = aux
                nc.vector.tensor_tensor(
                    out=p[:, 0::4, :],
                    in0=aux[:],
                    in1=dc[:, 0::4, :],
                    op=Alu.mult,
                )

                # ---- kf' = sum_k p_k: binary tree; last level on GpSimd ----
                t1 = spool.tile([P, 4, nt], bf16, tag="t1")
                nc.vector.tensor_tensor(
                    out=t1[:], in0=p[:, 0:4, :], in1=p[:, 4:8, :], op=Alu.add
                )
                t2 = spool.tile([P, 2, nt], bf16, tag="t2")
                nc.vector.tensor_tensor(
                    out=t2[:], in0=t1[:, 0:2, :], in1=t1[:, 2:4, :], op=Alu.add
                )
                kf = spool.tile([P, nt], bf16, tag="kf")
                nc.gpsimd.tensor_tensor(
                    out=kf[:], in0=t2[:, 0, :], in1=t2[:, 1, :], op=Alu.add
                )

                # ---- gate = relu(6 * kf') on ScalarE ----
                gate = spool.tile([P, nt], bf16, tag="gate")
                nc.scalar.activation(
                    out=gate[:], in_=kf[:], func=ActF.Relu, scale=6.0
                )

                # ---- og = d * gate (bcast over k): TT-mult 4x ----
                og = opool.tile([P, K, nt], bf16, tag="og")
                nc.vector.tensor_tensor(
                    out=og[:],
                    in0=dcb,
                    in1=_ap(gate[:], 0, [[0, K], [1, nt]]),
                    op=Alu.mult,
                )

                # ---- o2 = og + x: GpSimd takes 5 planes, DVE takes 3 ----
                o2 = opool.tile([P, K, nt], bf16, tag="o2")
                nc.gpsimd.tensor_tensor(
                    out=o2[:, 0:5, :], in0=og[:, 0:5, :], in1=xg[:, 0:5, :],
                    op=Alu.add,
                )
                nc.vector.tensor_tensor(
                    out=o2[:, 5:8, :], in0=og[:, 5:8, :], in1=xg[:, 5:8, :],
                    op=Alu.add,
                )

                nc.sync.dma_start(
                    out=out[gt * P : (gt + 1) * P, :, c * nt : (c + 1) * nt],
                    in_=o2[:],
                )

    _split_waits(nc)
    return nc


# Engine datapath structs (Matmult/TT/STT/Act/...) only carry ONE sync wait on
# TRN2 walrus; sequencer instructions (NoOp) can each carry one more.  Hoist
# surplus waits onto same-engine NoOps placed just before the instruction.
_SEQ_OK = set()  # every struct on this walrus takes at most ONE sync wait


def _split_waits(nc):
    nnop = 0
    for fn in nc.m.functions:
        for blk in fn.blocks:
            out = []
            for inst in blk.instructions:
                si = inst.sync_info
                if (
                    si is not None
                    and si.on_wait
                    and len(si.on_wait) > 1
                    and type(inst).__name__ not in _SEQ_OK
                ):
                    for w in si.on_wait[:-1]:
                        nop = mybir.InstNoOp(
                            name=f"{inst.name}-sw{nnop}",
                            opcode="NoOp",
                            engine=inst.engine,
                            sync_info=mybir.SyncInfo(on_wait=[w], on_update=[]),
                        )
                        nnop += 1
                        out.append(nop)
                    inst.sync_info = mybir.SyncInfo(
                        on_wait=[si.on_wait[-1]], on_update=list(si.on_update)
                    )
                out.append(inst)
            blk.instructions[:] = out
    return nc


_NC_CACHE = {}


def _get_nc(n_total=N, nt=256):
    key = (n_total, nt)
    if key not in _NC_CACHE:
        _NC_CACHE[key] = build_nc(n_total, nt)
    return _NC_CACHE[key]


def _to_bf16(a: np.ndarray) -> np.ndarray:
    import ml_dtypes

    return np.ascontiguousarray(a.astype(ml_dtypes.bfloat16))


def kernel(x: np.ndarray, W: np.ndarray) -> np.ndarray:
    assert x.shape == (B, F, K, N) and W.shape == (F, F)
    wt = _to_bf16(W.T.copy())
    x16 = _to_bf16(x)
    in_maps = [{"x": x16[b], "wt": wt} for b in range(B)]
    nc = _get_nc()
    res = run_bass_kernel_spmd(nc, in_maps, list(range(B)))
    return np.stack(
        [res.results[b]["out"].astype(np.float32) for b in range(B)], axis=0
    )


if __name__ == "__main__":
    xs = np.random.randn(B, F, K, N).astype(np.float32)
    Ws = (np.random.randn(F, F) / np.sqrt(F)).astype(np.float32)
    o = kernel(xs, Ws)
    print(o.shape, o.dtype)


# revision 5
# speedup vs baseline: 1.9512x; 1.0416x over previous
# Trainium2 Bass kernel for nn_LNKillingRelu: out = where(kf<=0, x, x + kf*d)
#   d  = einsum('fkn,gf->gkn', x, W)                      (per batch)
#   kf = einsum('fkn,kl,fln->fn', x, G, d)  broadcast over k
# G is the (constant) Killing-form Gram matrix of sl(3):
#   G[0,0]=G[4,4]=12, G[0,4]=G[4,0]=-6, G[1,3]=G[3,1]=G[2,6]=G[6,2]=G[5,7]=G[7,5]=6
# so with kf' = kf/6:
#   kf' = x0*(2d0-d4) + x4*(2d4-d0) + x1*d3 + x3*d1 + x2*d6 + x6*d2 + x5*d7 + x7*d5
#   out = x + relu(6*kf') * d
#
# v4: bf16, tuned to measured HW rates (rel err ~7e-3; harness gate 2e-2).
# Measured op rates that drive the layout (FD = free-dim elems @0.96GHz DVE):
#  - PE bf16 matmul 216ns/512-row when hot (4x over fp32)
#  - DVE TT MULT on fully-contiguous 1-dim APs: 4x mode (58+FD/4 cyc);
#    strided/2-dim/broadcast APs drop to 2x; ADD caps at 2x; STT 1x
#  - GpSimd shares an SBUF port with DVE: concurrent same-tile streaming
#    stretches DVE ops ~5x, so GpSimd only gets ops on tiles DVE is not
#    concurrently reading (t2/kf tree tail)
#  - ScalarE ~(224+FD)/1.2GHz: does the PSUM->SBUF d copy, 2*x scale, relu
# Therefore: products and og are 8 per-plane contiguous TT-mults each (4x),
# tree level 1 and the final +x are single DVE ADDs (2x), nt=512 halves the
# per-op fixed costs, PSUM is split into two 4-bank half-tiles (k 0-3 / 4-7).
#
# Sharding: data-parallel over batch B=8 -> one batch per NeuronCore (8 cores).
# W is replicated (host passes W^T in bf16 so lhsT chunks slice directly).

from contextlib import ExitStack

import numpy as np

import concourse.bass as bass
import concourse.mybir as mybir
import concourse.tile as tile
from concourse.bass_utils import run_bass_kernel_spmd

B, F, K, N = 8, 512, 8, 2048
P = 128
FT = F // P  # 4 channel tiles
KH = K // 2  # planes per PSUM half

f32 = mybir.dt.float32
bf16 = mybir.dt.bfloat16
Alu = mybir.AluOpType
ActF = mybir.ActivationFunctionType

# z-plane source in x for the pure-permutation planes: z_l = x[ZSRC[l]]
ZSRC = {1: 3, 2: 6, 3: 1, 5: 7, 6: 2, 7: 5}


def build_nc(n_total=N, nt=512):
    nch = n_total // nt
    # race detection chokes on the post-hoc wait-split NoOps (they lack the
    # rust pass's fake sem updates); correctness is validated vs reference.
    nc = bass.Bass(detect_race_conditions=False)
    x = nc.dram_tensor("x", [F, K, n_total], bf16, kind="ExternalInput")
    wt = nc.dram_tensor("wt", [F, F], bf16, kind="ExternalInput")  # W^T (f, g)
    out = nc.dram_tensor("out", [F, K, n_total], bf16, kind="ExternalOutput")

    with tile.TileContext(nc) as tc, ExitStack() as ctx:
        wpool = ctx.enter_context(tc.tile_pool(name="w", bufs=1))
        xpool = ctx.enter_context(tc.tile_pool(name="xc", bufs=2))
        papool = ctx.enter_context(tc.tile_pool(name="pda", bufs=1, space="PSUM"))
        pbpool = ctx.enter_context(tc.tile_pool(name="pdb", bufs=1, space="PSUM"))
        dcpool = ctx.enter_context(tc.tile_pool(name="dc", bufs=2))
        prpool = ctx.enter_context(tc.tile_pool(name="prod", bufs=2))
        spool = ctx.enter_context(tc.tile_pool(name="small", bufs=3))
        opool = ctx.enter_context(tc.tile_pool(name="og", bufs=2))

        # resident W^T tiles: wsb[ft][p, g] , f = ft*128+p
        wsb = []
        for ft in range(FT):
            w_t = wpool.tile([P, F], bf16, tag=f"w{ft}")
            nc.sync.dma_start(out=w_t[:], in_=wt[ft * P : (ft + 1) * P, :])
            wsb.append(w_t)

        # Walrus only allows ONE sync wait per Matmult (waits ride the
        # LDWEIGHTS struct).  Warmup matmuls make PE observe each W-DMA
        # semaphore individually so later matmuls never wait on W.
        warm = papool.tile([P, KH, nt], f32, tag="pda")
        for ft in range(FT):
            nc.tensor.matmul(
                warm[:, 0, 0:1], wsb[ft][:, 0:P], wsb[ft][:, 0:1], start=True, stop=True
            )

        nmm = (KH * nt) // 512  # 512-elem free chunks (one PSUM bank each)
        kper = max(512 // nt, 1)  # k planes per matmul chunk

        for c in range(nch):
            xcs = []
            for ft in range(FT):
                xt = xpool.tile([P, K, nt], bf16, tag=f"xc{ft}")
                nc.sync.dma_start(
                    out=xt[:],
                    in_=x[ft * P : (ft + 1) * P, :, c * nt : (c + 1) * nt],
                )
                xcs.append(xt)
            for gt in range(FT):
                xg = xcs[gt][:]  # [P, K, nt] bf16 x for this channel tile
                dc = dcpool.tile([P, K, nt], bf16, tag="dc")

                # ---- matmul halves: d[g, k-half, n-chunk] over f tiles ----
                for half, pool in ((0, papool), (1, pbpool)):
                    pd = pool.tile([P, KH, nt], f32, tag=("pda", "pdb")[half])
                    # Dummy first matmul absorbs the PSUM-slot-release wait so
                    # the first real one only waits on its x DMA (1-wait limit).
                    nc.tensor.matmul(
                        pd[:, 0, 0:1], wsb[0][:, 0:P], wsb[0][:, 0:1],
                        start=True, stop=True,
                    )
                    k0 = half * KH
                    for ft in range(FT):
                        for jj in range(nmm):
                            nc.tensor.matmul(
                                pd[:, jj * kper : (jj + 1) * kper, :],
                                wsb[ft][:, gt * P : (gt + 1) * P],
                                xcs[ft][:, k0 + jj * kper : k0 + (jj + 1) * kper, :],
                                start=(ft == 0),
                                stop=(ft == FT - 1),
                            )
                    # dc half = bf16(d): PSUM -> SBUF on ScalarE (frees PSUM)
                    nc.scalar.copy(out=dc[:, k0 : k0 + KH, :], in_=pd[:])

                # ---- x2 = 2*(x0, x4) on ScalarE (exact in bf16) ----
                x2 = spool.tile([P, 2, nt], bf16, tag="x2")
                nc.scalar.activation(
                    out=x2[:], in_=xg[:, 0::4, :], func=ActF.Copy, scale=2.0
                )

                # ---- aux = (2x0-x4, 2x4-x0): one DVE TT subtract (2x) ----
                aux = spool.tile([P, 2, nt], bf16, tag="aux")
                nc.vector.tensor_tensor(
                    out=aux[:], in0=x2[:], in1=xg[:, 4::-4, :], op=Alu.subtract
                )

                # ---- products p_l = z_l * d_l: 8 per-plane contiguous
                # TT-mults (each fully 1-dim -> DVE 4x mode) ----
                p = prpool.tile([P, K, nt], bf16, tag="p")
                nc.vector.tensor_tensor(
                    out=p[:, 0, :], in0=aux[:, 0, :], in1=dc[:, 0, :], op=Alu.mult
                )
                nc.vector.tensor_tensor(
                    out=p[:, 4, :], in0=aux[:, 1, :], in1=dc[:, 4, :], op=Alu.mult
                )
                for l, src in ZSRC.items():
                    nc.vector.tensor_tensor(
                        out=p[:, l, :], in0=xg[:, src, :], in1=dc[:, l, :],
                        op=Alu.mult,
                    )

                # ---- kf' = sum_k p_k: t1 on DVE; t2+kf tail on GpSimd
                # (GpSimd only touches t1/t2, which DVE is done with) ----
                t1 = spool.tile([P, 4, nt], bf16, tag="t1")
                nc.vector.tensor_tensor(
                    out=t1[:], in0=p[:, 0:4, :], in1=p[:, 4:8, :], op=Alu.add
                )
                t2 = spool.tile([P, 2, nt], bf16, tag="t2")
                nc.gpsimd.tensor_tensor(
                    out=t2[:], in0=t1[:, 0:2, :], in1=t1[:, 2:4, :], op=Alu.add
                )
                kf = spool.tile([P, nt], bf16, tag="kf")
                nc.gpsimd.tensor_tensor(
                    out=kf[:], in0=t2[:, 0, :], in1=t2[:, 1, :], op=Alu.add
                )

                # ---- gate = relu(6 * kf') on ScalarE ----
                gate = spool.tile([P, nt], bf16, tag="gate")
                nc.scalar.activation(
                    out=gate[:], in_=kf[:], func=ActF.Relu, scale=6.0
                )

                # ---- og_k = d_k * gate: 8 per-plane contiguous mults (4x) ----
                og = opool.tile([P, K, nt], bf16, tag="og")
                for l in range(K):
                    nc.vector.tensor_tensor(
                        out=og[:, l, :], in0=dc[:, l, :], in1=gate[:], op=Alu.mult
                    )

                # ---- o2 = og + x: one DVE ADD (2x) ----
                o2 = opool.tile([P, K, nt], bf16, tag="o2")
                nc.vector.tensor_tensor(
                    out=o2[:], in0=og[:], in1=xg, op=Alu.add
                )

                nc.sync.dma_start(
                    out=out[gt * P : (gt + 1) * P, :, c * nt : (c + 1) * nt],
                    in_=o2[:],
                )

    _split_waits(nc)
    return nc


# Engine datapath structs (Matmult/TT/STT/Act/...) only carry ONE sync wait on
# TRN2 walrus; sequencer instructions (NoOp) can each carry one more.  Hoist
# surplus waits onto same-engine NoOps placed just before the instruction.
_SEQ_OK = set()  # every struct on this walrus takes at most ONE sync wait


def _split_waits(nc):
    nnop = 0
    for fn in nc.m.functions:
        for blk in fn.blocks:
            out = []
            for inst in blk.instructions:
                si = inst.sync_info
                if (
                    si is not None
                    and si.on_wait
                    and len(si.on_wait) > 1
                    and type(inst).__name__ not in _SEQ_OK
                ):
                    for w in si.on_wait[:-1]:
                        nop = mybir.InstNoOp(
                            name=f"{inst.name}-sw{nnop}",
                            opcode="NoOp",
                            engine=inst.engine,
                            sync_info=mybir.SyncInfo(on_wait=[w], on_update=[]),
                        )
                        nnop += 1
                        out.append(nop)
                    inst.sync_info = mybir.SyncInfo(
                        on_wait=[si.on_wait[-1]], on_update=list(si.on_update)
                    )
                out.append(inst)
            blk.instructions[:] = out
    return nc


_NC_CACHE = {}


def _get_nc(n_total=N, nt=512):
    key = (n_total, nt)
    if key not in _NC_CACHE:
        _NC_CACHE[key] = build_nc(n_total, nt)
    return _NC_CACHE[key]


def _to_bf16(a: np.ndarray) -> np.ndarray:
    import ml_dtypes

    return np.ascontiguousarray(a.astype(ml_dtypes.bfloat16))


def kernel(x: np.ndarray, W: np.ndarray) -> np.ndarray:
    assert x.shape == (B, F, K, N) and W.shape == (F, F)
    wt = _to_bf16(W.T.copy())
    x16 = _to_bf16(x)
    in_maps = [{"x": x16[b], "wt": wt} for b in range(B)]
    nc = _get_nc()
    res = run_bass_kernel_spmd(nc, in_maps, list(range(B)))
    return np.stack(
        [res.results[b]["out"].astype(np.float32) for b in range(B)], axis=0
    )


if __name__ == "__main__":
    xs = np.random.randn(B, F, K, N).astype(np.float32)
    Ws = (np.random.randn(F, F) / np.sqrt(F)).astype(np.float32)
    o = kernel(xs, Ws)
    print(o.shape, o.dtype)


# revision 6
# speedup vs baseline: 2.1035x; 1.0780x over previous
# Trainium2 Bass kernel for nn_LNKillingRelu: out = where(kf<=0, x, x + kf*d)
#   d  = einsum('fkn,gf->gkn', x, W)                      (per batch)
#   kf = einsum('fkn,kl,fln->fn', x, G, d)  broadcast over k
# G is the (constant) Killing-form Gram matrix of sl(3):
#   G[0,0]=G[4,4]=12, G[0,4]=G[4,0]=-6, G[1,3]=G[3,1]=G[2,6]=G[6,2]=G[5,7]=G[7,5]=6
# so with kf' = kf/6:
#   kf' = x0*(2d0-d4) + x4*(2d4-d0) + x1*d3 + x3*d1 + x2*d6 + x6*d2 + x5*d7 + x7*d5
#   out = x + relu(6*kf') * d
#
# v5: bf16 (rel err ~7e-3; harness gate 2e-2), engineered to measured HW:
#  - k-planes live in SBUF in the PI order (1,3, 2,6, 5,7, 0,4): G's pair
#    swaps become an affine negative-stride AP, so the products need only
#    2 strided TT-mults + 1 contiguous one instead of per-plane ops.
#    Pair-DMAs (stride-2/4 plane pairs) load/store this layout directly.
#  - gate is materialized 4-wide by the ScalarE relu (broadcast read), so
#    og = d*gate is two fully-contiguous FD2048 TT-mults (DVE 4x mode).
#  - lag-2 software pipeline: og/o2/out-DMA of iteration i-2 are emitted in
#    iteration i, so the DVE never waits on the kf->relu ScalarE round-trip.
#  - final +x is split: DVE adds slots 0-3, GpSimd adds slots 4-7 (GpSimd
#    shares an SBUF port with DVE, so it only gets work whose tiles the DVE
#    is done streaming; the shared o2 tile serializes the two halves).
#  - PSUM is two 4-bank half-tiles (slots 0-3 / 4-7), each single-buffered;
#    ScalarE copies each half to SBUF bf16 right after its 16 matmuls.
#
# Sharding: data-parallel over batch B=8 -> one batch per NeuronCore (8 cores).
# W is replicated (host passes W^T in bf16 so lhsT chunks slice directly).

from contextlib import ExitStack

import numpy as np

import concourse.bass as bass
import concourse.mybir as mybir
import concourse.tile as tile
from concourse.bass_utils import run_bass_kernel_spmd

B, F, K, N = 8, 512, 8, 2048
P = 128
FT = F // P  # 4 channel tiles
KH = K // 2  # plane slots per PSUM half

f32 = mybir.dt.float32
bf16 = mybir.dt.bfloat16
Alu = mybir.AluOpType
ActF = mybir.ActivationFunctionType

# SBUF slot j holds k-plane PI[j]; G-pairs are adjacent, aux pair (0,4) last.
PI = (1, 3, 2, 6, 5, 7, 0, 4)
# (start, step) of the HBM plane pair backing slots (2j, 2j+1)
PAIRS = ((1, 2), (2, 4), (5, 2), (0, 4))


def _ap(base, off_elems, dims):
    """Raw AP from a base AP: keep partition dim, replace free dims."""
    return bass.AP(
        tensor=base.tensor,
        offset=base.offset + off_elems,
        ap=[base.ap[0]] + dims,
    )


def build_nc(n_total=N, nt=512):
    nch = n_total // nt
    # race detection chokes on the post-hoc wait-split NoOps (they lack the
    # rust pass's fake sem updates); correctness is validated vs reference.
    nc = bass.Bass(detect_race_conditions=False)
    x = nc.dram_tensor("x", [F, K, n_total], bf16, kind="ExternalInput")
    wt = nc.dram_tensor("wt", [F, F], bf16, kind="ExternalInput")  # W^T (f, g)
    out = nc.dram_tensor("out", [F, K, n_total], bf16, kind="ExternalOutput")

    with tile.TileContext(nc) as tc, ExitStack() as ctx:
        wpool = ctx.enter_context(tc.tile_pool(name="w", bufs=1))
        xpool = ctx.enter_context(tc.tile_pool(name="xc", bufs=2))
        papool = ctx.enter_context(tc.tile_pool(name="pda", bufs=1, space="PSUM"))
        pbpool = ctx.enter_context(tc.tile_pool(name="pdb", bufs=1, space="PSUM"))
        dcpool = ctx.enter_context(tc.tile_pool(name="dc", bufs=3))
        prpool = ctx.enter_context(tc.tile_pool(name="prod", bufs=2))
        s2pool = ctx.enter_context(tc.tile_pool(name="s2", bufs=2))
        s3pool = ctx.enter_context(tc.tile_pool(name="s3", bufs=3))
        opool = ctx.enter_context(tc.tile_pool(name="og", bufs=2))

        # resident W^T tiles: wsb[ft][p, g] , f = ft*128+p
        wsb = []
        for ft in range(FT):
            w_t = wpool.tile([P, F], bf16, tag=f"w{ft}")
            nc.sync.dma_start(out=w_t[:], in_=wt[ft * P : (ft + 1) * P, :])
            wsb.append(w_t)

        # Walrus only allows ONE sync wait per Matmult (waits ride the
        # LDWEIGHTS struct).  Warmup matmuls make PE observe each W-DMA
        # semaphore individually so later matmuls never wait on W.
        warm = papool.tile([P, KH, nt], f32, tag="pda")
        for ft in range(FT):
            nc.tensor.matmul(
                warm[:, 0, 0:1], wsb[ft][:, 0:P], wsb[ft][:, 0:1], start=True, stop=True
            )

        def emit_gate(st):
            # gate4 = relu(6*kf) replicated over 4 slots (broadcast read)
            gate4 = s3pool.tile([P, 4, nt], bf16, tag="gate4")
            nc.scalar.activation(
                out=gate4[:],
                in_=_ap(st["kf"], 0, [[0, 4], [1, nt]]),
                func=ActF.Relu,
                scale=6.0,
            )
            st["gate4"] = gate4

        def flush(st):
            # og = d * gate (two contiguous FD2048 mults, DVE 4x mode)
            dcb, xgb, g4, c, gt = st["dc"], st["xg"], st["gate4"], st["c"], st["gt"]
            oga = opool.tile([P, 4, nt], bf16, tag="oga")
            nc.vector.tensor_tensor(
                out=oga[:], in0=dcb[:, 0:4, :], in1=g4[:], op=Alu.mult
            )
            ogb = opool.tile([P, 4, nt], bf16, tag="ogb")
            nc.vector.tensor_tensor(
                out=ogb[:], in0=dcb[:, 4:8, :], in1=g4[:], op=Alu.mult
            )
            # o2 = og + x: DVE slots 0-3; GpSimd slots 4-7 (shared o2 tile
            # serializes GpSimd behind the DVE half -> no same-tile overlap)
            o2 = opool.tile([P, K, nt], bf16, tag="o2")
            nc.vector.tensor_tensor(
                out=o2[:, 0:4, :], in0=oga[:], in1=xgb[:, 0:4, :], op=Alu.add
            )
            nc.gpsimd.tensor_tensor(
                out=o2[:, 4:8, :], in0=ogb[:], in1=xgb[:, 4:8, :], op=Alu.add
            )
            gsl = slice(gt * P, (gt + 1) * P)
            chs = slice(c * nt, (c + 1) * nt)
            for j, (p0, ps) in enumerate(PAIRS):
                nc.sync.dma_start(
                    out=out[gsl, p0 : p0 + ps + 1 : ps, chs],
                    in_=o2[:, 2 * j : 2 * j + 2, :],
                )

        pending = []
        for c in range(nch):
            xcs = []
            for ft in range(FT):
                xt = xpool.tile([P, K, nt], bf16, tag=f"xc{ft}")
                fsl = slice(ft * P, (ft + 1) * P)
                chs = slice(c * nt, (c + 1) * nt)
                for j, (p0, ps) in enumerate(PAIRS):
                    nc.sync.dma_start(
                        out=xt[:, 2 * j : 2 * j + 2, :],
                        in_=x[fsl, p0 : p0 + ps + 1 : ps, chs],
                    )
                xcs.append(xt)
            for gt in range(FT):
                xg = xcs[gt][:]  # [P, K(slots), nt] bf16, PI order
                dc = dcpool.tile([P, K, nt], bf16, tag="dc")

                # ---- matmul halves -> PSUM, ScalarE copy to bf16 SBUF ----
                for half, pool in ((0, papool), (1, pbpool)):
                    pd = pool.tile([P, KH, nt], f32, tag=("pda", "pdb")[half])
                    # Dummy matmul absorbs the PSUM-slot-release wait
                    # (1-wait limit on Matmult structs).
                    nc.tensor.matmul(
                        pd[:, 0, 0:1], wsb[0][:, 0:P], wsb[0][:, 0:1],
                        start=True, stop=True,
                    )
                    k0 = half * KH
                    for ft in range(FT):
                        for jj in range(KH):
                            nc.tensor.matmul(
                                pd[:, jj, :],
                                wsb[ft][:, gt * P : (gt + 1) * P],
                                xcs[ft][:, k0 + jj, :],
                                start=(ft == 0),
                                stop=(ft == FT - 1),
                            )
                    nc.scalar.copy(out=dc[:, k0 : k0 + KH, :], in_=pd[:])
                    if half == 0:
                        # x2 = 2*(x0,x4) (slots 6,7; exact in bf16) -- emitted
                        # between the copies so it never stalls behind dcB's
                        # wait on the second matmul half.
                        x2 = s3pool.tile([P, 2, nt], bf16, tag="x2")
                        nc.scalar.activation(
                            out=x2[:], in_=xg[:, 6:8, :], func=ActF.Copy,
                            scale=2.0,
                        )

                # ---- flush iteration i-2 (gate long since ready) ----
                if len(pending) == 2:
                    flush(pending.pop(0))

                # ---- aux = (2x0-x4, 2x4-x0): TT sub, slots (7,6) reversed ----
                aux = s2pool.tile([P, 2, nt], bf16, tag="aux")
                nc.vector.tensor_tensor(
                    out=aux[:],
                    in0=x2[:],
                    in1=_ap(xg, 7 * nt, [[-nt, 2], [1, nt]]),
                    op=Alu.subtract,
                )

                # ---- products p_j = z*d per slot; pair-swapped reads ----
                p = prpool.tile([P, K, nt], bf16, tag="p")
                # slots 0-3 (pairs (1,3),(2,6)): in0 = x pair-swapped
                nc.vector.tensor_tensor(
                    out=_ap(p[:], 0, [[2 * nt, 2], [nt, 2], [1, nt]]),
                    in0=_ap(xg, nt, [[2 * nt, 2], [-nt, 2], [1, nt]]),
                    in1=_ap(dc[:], 0, [[2 * nt, 2], [nt, 2], [1, nt]]),
                    op=Alu.mult,
                )
                # slots 4-5 (pair (5,7))
                nc.vector.tensor_tensor(
                    out=_ap(p[:], 4 * nt, [[nt, 2], [1, nt]]),
                    in0=_ap(xg, 5 * nt, [[-nt, 2], [1, nt]]),
                    in1=_ap(dc[:], 4 * nt, [[nt, 2], [1, nt]]),
                    op=Alu.mult,
                )
                # slots 6-7 ((0,4) via aux): fully contiguous
                nc.vector.tensor_tensor(
                    out=p[:, 6:8, :], in0=aux[:], in1=dc[:, 6:8, :], op=Alu.mult
                )

                # ---- kf' = sum over slots: 3-level DVE add tree ----
                t1 = s2pool.tile([P, 4, nt], bf16, tag="t1")
                nc.vector.tensor_tensor(
                    out=t1[:], in0=p[:, 0:4, :], in1=p[:, 4:8, :], op=Alu.add
                )
                t2 = s2pool.tile([P, 2, nt], bf16, tag="t2")
                nc.vector.tensor_tensor(
                    out=t2[:], in0=t1[:, 0:2, :], in1=t1[:, 2:4, :], op=Alu.add
                )
                kf = s3pool.tile([P, nt], bf16, tag="kf")
                nc.vector.tensor_tensor(
                    out=kf[:], in0=t2[:, 0, :], in1=t2[:, 1, :], op=Alu.add
                )

                st = {"dc": dc, "xg": xg, "kf": kf, "c": c, "gt": gt}
                pending.append(st)
                # gate for the PREVIOUS iteration (its kf finished during the
                # previous step, so ScalarE never stalls here)
                if len(pending) >= 2:
                    emit_gate(pending[-2])

        emit_gate(pending[-1])
        for st in pending:
            flush(st)

    _split_waits(nc)
    return nc


# Engine datapath structs (Matmult/TT/STT/Act/...) only carry ONE sync wait on
# TRN2 walrus; sequencer instructions (NoOp) can each carry one more.  Hoist
# surplus waits onto same-engine NoOps placed just before the instruction.
_SEQ_OK = set()  # every struct on this walrus takes at most ONE sync wait


def _split_waits(nc):
    nnop = 0
    for fn in nc.m.functions:
        for blk in fn.blocks:
            out = []
            for inst in blk.instructions:
                si = inst.sync_info
                if (
                    si is not None
                    and si.on_wait
                    and len(si.on_wait) > 1
                    and type(inst).__name__ not in _SEQ_OK
                ):
                    for w in si.on_wait[:-1]:
                        nop = mybir.InstNoOp(
                            name=f"{inst.name}-sw{nnop}",
                            opcode="NoOp",
                            engine=inst.engine,
                            sync_info=mybir.SyncInfo(on_wait=[w], on_update=[]),
                        )
                        nnop += 1
                        out.append(nop)
                    inst.sync_info = mybir.SyncInfo(
                        on_wait=[si.on_wait[-1]], on_update=list(si.on_update)
                    )
                out.append(inst)
            blk.instructions[:] = out
    return nc


_NC_CACHE = {}


def _get_nc(n_total=N, nt=512):
    key = (n_total, nt)
    if key not in _NC_CACHE:
        _NC_CACHE[key] = build_nc(n_total, nt)
    return _NC_CACHE[key]


def _to_bf16(a: np.ndarray) -> np.ndarray:
    import ml_dtypes

    return np.ascontiguousarray(a.astype(ml_dtypes.bfloat16))


def kernel(x: np.ndarray, W: np.ndarray) -> np.ndarray:
    assert x.shape == (B, F, K, N) and W.shape == (F, F)
    wt = _to_bf16(W.T.copy())
    x16 = _to_bf16(x)
    in_maps = [{"x": x16[b], "wt": wt} for b in range(B)]
    nc = _get_nc()
    res = run_bass_kernel_spmd(nc, in_maps, list(range(B)))
    return np.stack(
        [res.results[b]["out"].astype(np.float32) for b in range(B)], axis=0
    )


if __name__ == "__main__":
    xs = np.random.randn(B, F, K, N).astype(np.float32)
    Ws = (np.random.randn(F, F) / np.sqrt(F)).astype(np.float32)
    o = kernel(xs, Ws)
    print(o.shape, o.dtype)


# revision 8
# speedup vs baseline: 2.5868x; 1.2297x over previous
# Trainium2 Bass kernel for nn_LNKillingRelu: out = where(kf<=0, x, x + kf*d)
#   d  = einsum('fkn,gf->gkn', x, W)                      (per batch)
#   kf = einsum('fkn,kl,fln->fn', x, G, d)  broadcast over k
# G is the (constant) Killing-form Gram matrix of sl(3):
#   G[0,0]=G[4,4]=12, G[0,4]=G[4,0]=-6, G[1,3]=G[3,1]=G[2,6]=G[6,2]=G[5,7]=G[7,5]=6
# so with kf' = kf/6:
#   kf' = x0*(2d0-d4) + x4*(2d4-d0) + x1*d3 + x3*d1 + x2*d6 + x6*d2 + x5*d7 + x7*d5
#   out = x + relu(6*kf') * d
#
# v5: f16 (rel err ~7e-3; harness gate 2e-2), engineered to measured HW:
#  - k-planes live in SBUF in the PI order (1,3, 2,6, 5,7, 0,4): G's pair
#    swaps become an affine negative-stride AP, so the products need only
#    2 strided TT-mults + 1 contiguous one instead of per-plane ops.
#    Pair-DMAs (stride-2/4 plane pairs) load/store this layout directly.
#  - gate is materialized 4-wide by the ScalarE relu (broadcast read), so
#    og = d*gate is two fully-contiguous FD2048 TT-mults (DVE 4x mode).
#  - lag-2 software pipeline: og/o2/out-DMA of iteration i-2 are emitted in
#    iteration i, so the DVE never waits on the kf->relu ScalarE round-trip.
#  - final +x is split: DVE adds slots 0-3, GpSimd adds slots 4-7 (GpSimd
#    shares an SBUF port with DVE, so it only gets work whose tiles the DVE
#    is done streaming; the shared o2 tile serializes the two halves).
#  - PSUM is two 4-bank half-tiles (slots 0-3 / 4-7), each single-buffered;
#    ScalarE copies each half to SBUF f16 right after its 16 matmuls.
#
# Sharding: data-parallel over batch B=8 -> one batch per NeuronCore (8 cores).
# W is replicated (host passes W^T in f16 so lhsT chunks slice directly).

from contextlib import ExitStack

import numpy as np

import concourse.bass as bass
import concourse.mybir as mybir
import concourse.tile as tile
from concourse.bass_utils import run_bass_kernel_spmd

B, F, K, N = 8, 512, 8, 2048
P = 128
FT = F // P  # 4 channel tiles
KH = K // 2  # plane slots per PSUM half

f32 = mybir.dt.float32
f16 = mybir.dt.float16
Alu = mybir.AluOpType
ActF = mybir.ActivationFunctionType

# SBUF slot j holds k-plane PI[j]; G-pairs are adjacent, aux pair (0,4) last.
PI = (1, 3, 2, 6, 5, 7, 0, 4)
# (start, step) of the HBM plane pair backing slots (2j, 2j+1)
PAIRS = ((1, 2), (2, 4), (5, 2), (0, 4))


def _ap(base, off_elems, dims):
    """Raw AP from a base AP: keep partition dim, replace free dims."""
    return bass.AP(
        tensor=base.tensor,
        offset=base.offset + off_elems,
        ap=[base.ap[0]] + dims,
    )


def build_nc(n_total=N, nt=512):
    nch = n_total // nt
    # race detection chokes on the post-hoc wait-split NoOps (they lack the
    # rust pass's fake sem updates); correctness is validated vs reference.
    nc = bass.Bass(detect_race_conditions=False)
    x = nc.dram_tensor("x", [F, K, n_total], f16, kind="ExternalInput")
    wt = nc.dram_tensor("wt", [F, F], f16, kind="ExternalInput")  # W^T (f, g)
    out = nc.dram_tensor("out", [F, K, n_total], f16, kind="ExternalOutput")

    with tile.TileContext(nc) as tc, ExitStack() as ctx:
        wpool = ctx.enter_context(tc.tile_pool(name="w", bufs=1))
        xpool = ctx.enter_context(tc.tile_pool(name="xc", bufs=2))
        papool = ctx.enter_context(tc.tile_pool(name="pda", bufs=1, space="PSUM"))
        pbpool = ctx.enter_context(tc.tile_pool(name="pdb", bufs=1, space="PSUM"))
        dcpool = ctx.enter_context(tc.tile_pool(name="dc", bufs=3))
        prpool = ctx.enter_context(tc.tile_pool(name="prod", bufs=2))
        s2pool = ctx.enter_context(tc.tile_pool(name="s2", bufs=2))
        s3pool = ctx.enter_context(tc.tile_pool(name="s3", bufs=3))
        opool = ctx.enter_context(tc.tile_pool(name="og", bufs=2))

        # resident W^T tiles: wsb[ft][p, g] , f = ft*128+p
        wsb = []
        for ft in range(FT):
            w_t = wpool.tile([P, F], f16, tag=f"w{ft}")
            nc.sync.dma_start(out=w_t[:], in_=wt[ft * P : (ft + 1) * P, :])
            wsb.append(w_t)

        # Walrus only allows ONE sync wait per Matmult (waits ride the
        # LDWEIGHTS struct).  Warmup matmuls make PE observe each W-DMA
        # semaphore individually so later matmuls never wait on W.
        warm = papool.tile([P, KH, nt], f32, tag="pda")
        for ft in range(FT):
            nc.tensor.matmul(
                warm[:, 0, 0:1], wsb[ft][:, 0:P], wsb[ft][:, 0:1], start=True, stop=True
            )

        def emit_gate(st):
            # gate4 = relu(6*kf) replicated over 4 slots (broadcast read)
            gate4 = s3pool.tile([P, 4, nt], f16, tag="gate4")
            nc.scalar.activation(
                out=gate4[:],
                in_=_ap(st["kf"], 0, [[0, 4], [1, nt]]),
                func=ActF.Relu,
                scale=6.0,
            )
            st["gate4"] = gate4

        def flush(st):
            # og = d * gate4-read-twice (one FD4096 TT-mult, 2x mode);
            # o2 = og + x (one FD4096 TT-add, 2x).  All DVE: GpSimd stalls
            # the DVE via the shared SBUF port, so it gets nothing.
            dcb, xgb, g4, c, gt = st["dc"], st["xg"], st["gate4"], st["c"], st["gt"]
            h = 4 * nt
            og = opool.tile([P, K, nt], f16, tag="og")
            nc.vector.tensor_tensor(
                out=_ap(og[:], 0, [[h, 2], [1, h]]),
                in0=_ap(dcb[:], 0, [[h, 2], [1, h]]),
                in1=_ap(g4, 0, [[0, 2], [1, h]]),
                op=Alu.mult,
            )
            o2 = opool.tile([P, K, nt], f16, tag="o2")
            nc.vector.tensor_tensor(
                out=o2[:], in0=og[:], in1=xgb, op=Alu.add
            )
            gsl = slice(gt * P, (gt + 1) * P)
            chs = slice(c * nt, (c + 1) * nt)
            for j, (p0, ps) in enumerate(PAIRS):
                nc.sync.dma_start(
                    out=out[gsl, p0 : p0 + ps + 1 : ps, chs],
                    in_=o2[:, 2 * j : 2 * j + 2, :],
                )

        pending = []
        for c in range(nch):
            xcs = []
            for ft in range(FT):
                xt = xpool.tile([P, K, nt], f16, tag=f"xc{ft}")
                fsl = slice(ft * P, (ft + 1) * P)
                chs = slice(c * nt, (c + 1) * nt)
                for j, (p0, ps) in enumerate(PAIRS):
                    nc.sync.dma_start(
                        out=xt[:, 2 * j : 2 * j + 2, :],
                        in_=x[fsl, p0 : p0 + ps + 1 : ps, chs],
                    )
                xcs.append(xt)
            for gt in range(FT):
                xg = xcs[gt][:]  # [P, K(slots), nt] f16, PI order
                dc = dcpool.tile([P, K, nt], f16, tag="dc")

                # ---- matmul halves -> PSUM, ScalarE copy to f16 SBUF ----
                for half, pool in ((0, papool), (1, pbpool)):
                    pd = pool.tile([P, KH, nt], f32, tag=("pda", "pdb")[half])
                    # Dummy matmul absorbs the PSUM-slot-release wait
                    # (1-wait limit on Matmult structs).
                    nc.tensor.matmul(
                        pd[:, 0, 0:1], wsb[0][:, 0:P], wsb[0][:, 0:1],
                        start=True, stop=True,
                    )
                    k0 = half * KH
                    for ft in range(FT):
                        for jj in range(KH):
                            nc.tensor.matmul(
                                pd[:, jj, :],
                                wsb[ft][:, gt * P : (gt + 1) * P],
                                xcs[ft][:, k0 + jj, :],
                                start=(ft == 0),
                                stop=(ft == FT - 1),
                            )
                    nc.scalar.copy(out=dc[:, k0 : k0 + KH, :], in_=pd[:])
                    if half == 0:
                        # x2 = 2*(x0,x4) (slots 6,7; exact in f16) -- emitted
                        # between the copies so it never stalls behind dcB's
                        # wait on the second matmul half.
                        x2 = s3pool.tile([P, 2, nt], f16, tag="x2")
                        nc.scalar.activation(
                            out=x2[:], in_=xg[:, 6:8, :], func=ActF.Copy,
                            scale=2.0,
                        )

                # ---- flush iteration i-2 (gate long since ready) ----
                if len(pending) == 2:
                    flush(pending.pop(0))

                # ---- aux = (2x0-x4, 2x4-x0): TT sub, slots (7,6) reversed ----
                aux = s2pool.tile([P, 2, nt], f16, tag="aux")
                nc.vector.tensor_tensor(
                    out=aux[:],
                    in0=x2[:],
                    in1=_ap(xg, 7 * nt, [[-nt, 2], [1, nt]]),
                    op=Alu.subtract,
                )

                # ---- products p_j = z*d per slot; pair-swapped reads ----
                p = prpool.tile([P, K, nt], f16, tag="p")
                # slots 0-3 (pairs (1,3),(2,6)): in0 = x pair-swapped
                nc.vector.tensor_tensor(
                    out=_ap(p[:], 0, [[2 * nt, 2], [nt, 2], [1, nt]]),
                    in0=_ap(xg, nt, [[2 * nt, 2], [-nt, 2], [1, nt]]),
                    in1=_ap(dc[:], 0, [[2 * nt, 2], [nt, 2], [1, nt]]),
                    op=Alu.mult,
                )
                # slots 4-5 (pair (5,7))
                nc.vector.tensor_tensor(
                    out=_ap(p[:], 4 * nt, [[nt, 2], [1, nt]]),
                    in0=_ap(xg, 5 * nt, [[-nt, 2], [1, nt]]),
                    in1=_ap(dc[:], 4 * nt, [[nt, 2], [1, nt]]),
                    op=Alu.mult,
                )
                # slots 6-7 ((0,4) via aux): fully contiguous
                nc.vector.tensor_tensor(
                    out=p[:, 6:8, :], in0=aux[:], in1=dc[:, 6:8, :], op=Alu.mult
                )

                # ---- kf' = sum over slots: 3-level DVE add tree ----
                t1 = s2pool.tile([P, 4, nt], f16, tag="t1")
                nc.vector.tensor_tensor(
                    out=t1[:], in0=p[:, 0:4, :], in1=p[:, 4:8, :], op=Alu.add
                )
                t2 = s2pool.tile([P, 2, nt], f16, tag="t2")
                nc.vector.tensor_tensor(
                    out=t2[:], in0=t1[:, 0:2, :], in1=t1[:, 2:4, :], op=Alu.add
                )
                kf = s3pool.tile([P, nt], f16, tag="kf")
                nc.vector.tensor_tensor(
                    out=kf[:], in0=t2[:, 0, :], in1=t2[:, 1, :], op=Alu.add
                )

                st = {"dc": dc, "xg": xg, "kf": kf, "c": c, "gt": gt}
                pending.append(st)
                # gate for the PREVIOUS iteration (its kf finished during the
                # previous step, so ScalarE never stalls here)
                if len(pending) >= 2:
                    emit_gate(pending[-2])

        emit_gate(pending[-1])
        for st in pending:
            flush(st)

    _split_waits(nc)
    return nc


# Engine datapath structs (Matmult/TT/STT/Act/...) only carry ONE sync wait on
# TRN2 walrus; sequencer instructions (NoOp) can each carry one more.  Hoist
# surplus waits onto same-engine NoOps placed just before the instruction.
_SEQ_OK = set()  # every struct on this walrus takes at most ONE sync wait


def _split_waits(nc):
    nnop = 0
    for fn in nc.m.functions:
        for blk in fn.blocks:
            out = []
            for inst in blk.instructions:
                si = inst.sync_info
                if (
                    si is not None
                    and si.on_wait
                    and len(si.on_wait) > 1
                    and type(inst).__name__ not in _SEQ_OK
                ):
                    for w in si.on_wait[:-1]:
                        nop = mybir.InstNoOp(
                            name=f"{inst.name}-sw{nnop}",
                            opcode="NoOp",
                            engine=inst.engine,
                            sync_info=mybir.SyncInfo(on_wait=[w], on_update=[]),
                        )
                        nnop += 1
                        out.append(nop)
                    inst.sync_info = mybir.SyncInfo(
                        on_wait=[si.on_wait[-1]], on_update=list(si.on_update)
                    )
                out.append(inst)
            blk.instructions[:] = out
    return nc


_NC_CACHE = {}


def _get_nc(n_total=N, nt=512):
    key = (n_total, nt)
    if key not in _NC_CACHE:
        _NC_CACHE[key] = build_nc(n_total, nt)
    return _NC_CACHE[key]


def _to_f16(a: np.ndarray) -> np.ndarray:
    return np.ascontiguousarray(a.astype(np.float16))


def kernel(x: np.ndarray, W: np.ndarray) -> np.ndarray:
    assert x.shape == (B, F, K, N) and W.shape == (F, F)
    wt = _to_f16(W.T.copy())
    x16 = _to_f16(x)
    in_maps = [{"x": x16[b], "wt": wt} for b in range(B)]
    nc = _get_nc()
    res = run_bass_kernel_spmd(nc, in_maps, list(range(B)))
    return np.stack(
        [res.results[b]["out"].astype(np.float32) for b in range(B)], axis=0
    )


if __name__ == "__main__":
    xs = np.random.randn(B, F, K, N).astype(np.float32)
    Ws = (np.random.randn(F, F) / np.sqrt(F)).astype(np.float32)
    o = kernel(xs, Ws)
    print(o.shape, o.dtype)


# revision 9
# speedup vs baseline: 2.5870x; 1.0001x over previous
# Trainium2 Bass kernel for nn_LNKillingRelu: out = where(kf<=0, x, x + kf*d)
#   d  = einsum('fkn,gf->gkn', x, W)                      (per batch)
#   kf = einsum('fkn,kl,fln->fn', x, G, d)  broadcast over k
# G is the (constant) Killing-form Gram matrix of sl(3):
#   G[0,0]=G[4,4]=12, G[0,4]=G[4,0]=-6, G[1,3]=G[3,1]=G[2,6]=G[6,2]=G[5,7]=G[7,5]=6
# so with kf' = kf/6:
#   kf' = x0*(2d0-d4) + x4*(2d4-d0) + x1*d3 + x3*d1 + x2*d6 + x6*d2 + x5*d7 + x7*d5
#   out = x + relu(6*kf') * d
#
# v5: f16 (rel err ~7e-3; harness gate 2e-2), engineered to measured HW:
#  - k-planes live in SBUF in the PI order (1,3, 2,6, 5,7, 0,4): G's pair
#    swaps become an affine negative-stride AP, so the products need only
#    2 strided TT-mults + 1 contiguous one instead of per-plane ops.
#    Pair-DMAs (stride-2/4 plane pairs) load/store this layout directly.
#  - gate is materialized 4-wide by the ScalarE relu (broadcast read), so
#    og = d*gate is two fully-contiguous FD2048 TT-mults (DVE 4x mode).
#  - lag-2 software pipeline: og/o2/out-DMA of iteration i-2 are emitted in
#    iteration i, so the DVE never waits on the kf->relu ScalarE round-trip.
#  - final +x is split: DVE adds slots 0-3, GpSimd adds slots 4-7 (GpSimd
#    shares an SBUF port with DVE, so it only gets work whose tiles the DVE
#    is done streaming; the shared o2 tile serializes the two halves).
#  - PSUM is two 4-bank half-tiles (slots 0-3 / 4-7), each single-buffered;
#    ScalarE copies each half to SBUF f16 right after its 16 matmuls.
#
# Sharding: data-parallel over batch B=8 -> one batch per NeuronCore (8 cores).
# W is replicated (host passes W^T in f16 so lhsT chunks slice directly).

from contextlib import ExitStack

import numpy as np

import concourse.bass as bass
import concourse.mybir as mybir
import concourse.tile as tile
from concourse.bass_utils import run_bass_kernel_spmd

B, F, K, N = 8, 512, 8, 2048
P = 128
FT = F // P  # 4 channel tiles
KH = K // 2  # plane slots per PSUM half

f32 = mybir.dt.float32
f16 = mybir.dt.float16
Alu = mybir.AluOpType
ActF = mybir.ActivationFunctionType

# SBUF slot j holds k-plane PI[j]; G-pairs are adjacent, aux pair (0,4) last.
PI = (1, 3, 2, 6, 5, 7, 0, 4)
# (start, step) of the HBM plane pair backing slots (2j, 2j+1)
PAIRS = ((1, 2), (2, 4), (5, 2), (0, 4))


def _ap(base, off_elems, dims):
    """Raw AP from a base AP: keep partition dim, replace free dims."""
    return bass.AP(
        tensor=base.tensor,
        offset=base.offset + off_elems,
        ap=[base.ap[0]] + dims,
    )


def build_nc(n_total=N, nt=512):
    nch = n_total // nt
    # race detection chokes on the post-hoc wait-split NoOps (they lack the
    # rust pass's fake sem updates); correctness is validated vs reference.
    nc = bass.Bass(detect_race_conditions=False)
    x = nc.dram_tensor("x", [F, K, n_total], f16, kind="ExternalInput")
    wt = nc.dram_tensor("wt", [F, F], f16, kind="ExternalInput")  # W^T (f, g)
    out = nc.dram_tensor("out", [F, K, n_total], f16, kind="ExternalOutput")

    with tile.TileContext(nc) as tc, ExitStack() as ctx:
        wpool = ctx.enter_context(tc.tile_pool(name="w", bufs=1))
        xpool = ctx.enter_context(tc.tile_pool(name="xc", bufs=2))
        papool = ctx.enter_context(tc.tile_pool(name="pda", bufs=1, space="PSUM"))
        pbpool = ctx.enter_context(tc.tile_pool(name="pdb", bufs=1, space="PSUM"))
        dcpool = ctx.enter_context(tc.tile_pool(name="dc", bufs=3))
        prpool = ctx.enter_context(tc.tile_pool(name="prod", bufs=2))
        s2pool = ctx.enter_context(tc.tile_pool(name="s2", bufs=2))
        s3pool = ctx.enter_context(tc.tile_pool(name="s3", bufs=3))
        opool = ctx.enter_context(tc.tile_pool(name="og", bufs=2))

        # resident W^T tiles: wsb[ft][p, g] , f = ft*128+p
        wsb = []
        for ft in range(FT):
            w_t = wpool.tile([P, F], f16, tag=f"w{ft}")
            nc.sync.dma_start(out=w_t[:], in_=wt[ft * P : (ft + 1) * P, :])
            wsb.append(w_t)

        # Walrus only allows ONE sync wait per Matmult (waits ride the
        # LDWEIGHTS struct).  Warmup matmuls make PE observe each W-DMA
        # semaphore individually so later matmuls never wait on W.
        warm = papool.tile([P, KH, nt], f32, tag="pda")
        for ft in range(FT):
            nc.tensor.matmul(
                warm[:, 0, 0:1], wsb[ft][:, 0:P], wsb[ft][:, 0:1], start=True, stop=True
            )

        def emit_gate(st):
            # gate4 = relu(6*kf) replicated over 4 slots (broadcast read)
            gate4 = s3pool.tile([P, 4, nt], f16, tag="gate4")
            nc.scalar.activation(
                out=gate4[:],
                in_=_ap(st["kf"], 0, [[0, 4], [1, nt]]),
                func=ActF.Relu,
                scale=6.0,
            )
            st["gate4"] = gate4

        def flush(st):
            # og = d * gate4-read-twice (one FD4096 TT-mult, 2x mode);
            # o2 = og + x (one FD4096 TT-add, 2x).  All DVE: GpSimd stalls
            # the DVE via the shared SBUF port, so it gets nothing.
            dcb, xgb, g4, c, gt = st["dc"], st["xg"], st["gate4"], st["c"], st["gt"]
            h = 4 * nt
            og = opool.tile([P, K, nt], f16, tag="og")
            nc.vector.tensor_tensor(
                out=_ap(og[:], 0, [[h, 2], [1, h]]),
                in0=_ap(dcb[:], 0, [[h, 2], [1, h]]),
                in1=_ap(g4, 0, [[0, 2], [1, h]]),
                op=Alu.mult,
            )
            o2 = opool.tile([P, K, nt], f16, tag="o2")
            nc.vector.tensor_tensor(
                out=o2[:], in0=og[:], in1=xgb, op=Alu.add
            )
            gsl = slice(gt * P, (gt + 1) * P)
            chs = slice(c * nt, (c + 1) * nt)
            for j, (p0, ps) in enumerate(PAIRS):
                nc.sync.dma_start(
                    out=out[gsl, p0 : p0 + ps + 1 : ps, chs],
                    in_=o2[:, 2 * j : 2 * j + 2, :],
                )

        pending = []   # iters awaiting og/o2 flush (lag 2)
        prev = None    # iter i-1: dcB + elementwise lagged one step

        def emit_tail(pv):
            # DVE elementwise for the previous iteration (its dcA landed
            # last step, its dcB at the head of this one -> no DVE stalls)
            xg, dc = pv["xg"], pv["dc"]
            aux = s2pool.tile([P, 2, nt], f16, tag="aux")
            nc.vector.tensor_tensor(
                out=aux[:],
                in0=pv["x2"][:],
                in1=_ap(xg, 7 * nt, [[-nt, 2], [1, nt]]),
                op=Alu.subtract,
            )
            p = prpool.tile([P, K, nt], f16, tag="p")
            # slots 0-3 (pairs (1,3),(2,6)): in0 = x pair-swapped
            nc.vector.tensor_tensor(
                out=_ap(p[:], 0, [[2 * nt, 2], [nt, 2], [1, nt]]),
                in0=_ap(xg, nt, [[2 * nt, 2], [-nt, 2], [1, nt]]),
                in1=_ap(dc[:], 0, [[2 * nt, 2], [nt, 2], [1, nt]]),
                op=Alu.mult,
            )
            # slots 4-5 (pair (5,7))
            nc.vector.tensor_tensor(
                out=_ap(p[:], 4 * nt, [[nt, 2], [1, nt]]),
                in0=_ap(xg, 5 * nt, [[-nt, 2], [1, nt]]),
                in1=_ap(dc[:], 4 * nt, [[nt, 2], [1, nt]]),
                op=Alu.mult,
            )
            # slots 6-7 ((0,4) via aux): fully contiguous
            nc.vector.tensor_tensor(
                out=p[:, 6:8, :], in0=aux[:], in1=dc[:, 6:8, :], op=Alu.mult
            )
            t1 = s2pool.tile([P, 4, nt], f16, tag="t1")
            nc.vector.tensor_tensor(
                out=t1[:], in0=p[:, 0:4, :], in1=p[:, 4:8, :], op=Alu.add
            )
            t2 = s2pool.tile([P, 2, nt], f16, tag="t2")
            nc.vector.tensor_tensor(
                out=t2[:], in0=t1[:, 0:2, :], in1=t1[:, 2:4, :], op=Alu.add
            )
            kf = s3pool.tile([P, nt], f16, tag="kf")
            nc.vector.tensor_tensor(
                out=kf[:], in0=t2[:, 0, :], in1=t2[:, 1, :], op=Alu.add
            )
            pv["kf"] = kf

        for c in range(nch):
            xcs = []
            for ft in range(FT):
                xt = xpool.tile([P, K, nt], f16, tag=f"xc{ft}")
                fsl = slice(ft * P, (ft + 1) * P)
                chs = slice(c * nt, (c + 1) * nt)
                for j, (p0, ps) in enumerate(PAIRS):
                    nc.sync.dma_start(
                        out=xt[:, 2 * j : 2 * j + 2, :],
                        in_=x[fsl, p0 : p0 + ps + 1 : ps, chs],
                    )
                xcs.append(xt)
            for gt in range(FT):
                xg = xcs[gt][:]  # [P, K(slots), nt] f16, PI order
                dc = dcpool.tile([P, K, nt], f16, tag="dc")

                # Scalar head: dcB of the PREVIOUS iter (its matmuls are
                # long done -> ScalarE starts the step immediately)
                if prev is not None:
                    nc.scalar.copy(
                        out=prev["dc"][:, KH:K, :], in_=prev["pdB"][:]
                    )
                # x2 = 2*(x0,x4) (slots 6,7; exact in f16) for THIS iter
                x2 = s3pool.tile([P, 2, nt], f16, tag="x2")
                nc.scalar.activation(
                    out=x2[:], in_=xg[:, 6:8, :], func=ActF.Copy, scale=2.0
                )

                # ---- matmul halves -> PSUM ----
                pds = []
                for half, pool in ((0, papool), (1, pbpool)):
                    pd = pool.tile([P, KH, nt], f32, tag=("pda", "pdb")[half])
                    # Dummy matmul absorbs the PSUM-slot-release wait
                    # (1-wait limit on Matmult structs).
                    nc.tensor.matmul(
                        pd[:, 0, 0:1], wsb[0][:, 0:P], wsb[0][:, 0:1],
                        start=True, stop=True,
                    )
                    k0 = half * KH
                    for ft in range(FT):
                        for jj in range(KH):
                            nc.tensor.matmul(
                                pd[:, jj, :],
                                wsb[ft][:, gt * P : (gt + 1) * P],
                                xcs[ft][:, k0 + jj, :],
                                start=(ft == 0),
                                stop=(ft == FT - 1),
                            )
                    pds.append(pd)
                    if half == 0:
                        # dcA for THIS iter (right after matmul half A)
                        nc.scalar.copy(out=dc[:, 0:KH, :], in_=pd[:])

                # ---- DVE: elementwise for iter i-1, then flush i-2 ----
                if prev is not None:
                    emit_tail(prev)
                    pending.append(prev)
                    emit_gate(prev)
                if len(pending) == 2:
                    flush(pending.pop(0))

                prev = {"dc": dc, "xg": xg, "x2": x2, "pdB": pds[1],
                        "c": c, "gt": gt}

        # drain: dcB + elementwise + gate for the last iter, then flush all
        nc.scalar.copy(out=prev["dc"][:, KH:K, :], in_=prev["pdB"][:])
        emit_tail(prev)
        pending.append(prev)
        emit_gate(prev)
        for st in pending:
            flush(st)

    _split_waits(nc)
    return nc


# Engine datapath structs (Matmult/TT/STT/Act/...) only carry ONE sync wait on
# TRN2 walrus; sequencer instructions (NoOp) can each carry one more.  Hoist
# surplus waits onto same-engine NoOps placed just before the instruction.
_SEQ_OK = set()  # every struct on this walrus takes at most ONE sync wait


def _split_waits(nc):
    nnop = 0
    for fn in nc.m.functions:
        for blk in fn.blocks:
            out = []
            for inst in blk.instructions:
                si = inst.sync_info
                if (
                    si is not None
                    and si.on_wait
                    and len(si.on_wait) > 1
                    and type(inst).__name__ not in _SEQ_OK
                ):
                    for w in si.on_wait[:-1]:
                        nop = mybir.InstNoOp(
                            name=f"{inst.name}-sw{nnop}",
                            opcode="NoOp",
                            engine=inst.engine,
                            sync_info=mybir.SyncInfo(on_wait=[w], on_update=[]),
                        )
                        nnop += 1
                        out.append(nop)
                    inst.sync_info = mybir.SyncInfo(
                        on_wait=[si.on_wait[-1]], on_update=list(si.on_update)
                    )
                out.append(inst)
            blk.instructions[:] = out
    return nc


_NC_CACHE = {}


def _get_nc(n_total=N, nt=512):
    key = (n_total, nt)
    if key not in _NC_CACHE:
        _NC_CACHE[key] = build_nc(n_total, nt)
    return _NC_CACHE[key]


def _to_f16(a: np.ndarray) -> np.ndarray:
    return np.ascontiguousarray(a.astype(np.float16))


def kernel(x: np.ndarray, W: np.ndarray) -> np.ndarray:
    assert x.shape == (B, F, K, N) and W.shape == (F, F)
    wt = _to_f16(W.T.copy())
    x16 = _to_f16(x)
    in_maps = [{"x": x16[b], "wt": wt} for b in range(B)]
    nc = _get_nc()
    res = run_bass_kernel_spmd(nc, in_maps, list(range(B)))
    return np.stack(
        [res.results[b]["out"].astype(np.float32) for b in range(B)], axis=0
    )


if __name__ == "__main__":
    xs = np.random.randn(B, F, K, N).astype(np.float32)
    Ws = (np.random.randn(F, F) / np.sqrt(F)).astype(np.float32)
    o = kernel(xs, Ws)
    print(o.shape, o.dtype)
